# revision 2
# baseline (speedup 1.0000x reference)
"""Trainium2 Bass kernel for nn_Model_39676907882504.

Math: qk = (q @ k^T)/8 has shape [1,2048,1,1]; after the transposes it is
[2048,1,1,1], and softmax over the trailing size-1 axis is exactly 1.0
regardless of qk (exp(x-max)/sum == 1/1 bit-exactly). The final matmul
[S,Q,B,Q] @ [B,S,Q,D] with attn_weight == 1 therefore reduces to
broadcasting `value` across a new leading dim:

    output[i, j, 0, :] = value[0, j, 0, :]   for all i in [0, 2048)

i.e. a 512KB -> 1GiB broadcast copy.  Pure memory-regime kernel.

Sharding (per the hint): leading output dim (2048 rows) split across the
8 cores, 256 rows/core; value replicated.

Per-core plan (viewing the 128MiB shard as 4096 sub-rows of 32KB; sub-row
d = output row d//16, value chunk d%16):

  * One 4MiB load of a host-pre-tiled value [128, 8192] (partition q =
    chunk q%16) replaces the baseline's 8 serialized 512KB loads.
  * 32 store blocks of 128 sub-rows each.  Trace analysis showed SDMA
    engine 15 sustains only ~21.4 GB/s vs ~26.9 GB/s for engines 0-14
    (port 15 = partitions 92-95/124-127), gating the whole kernel.  To
    rebalance, 13 blocks are stored 124 partitions wide (port 15 gets 4
    descriptors instead of 8); the 4 missing sub-rows per light block
    (chunks 12-15) are covered by small makeup stores out of a second
    tile whose partition groups sit on 13 distinct fast ports.
  * Stores split across both HW-DGE queues (SP + Activation).

Per-port descriptor budget: port 15 ~212 descs x ~1.53us, fast ports
~272 x ~1.22us -> ~331us span vs 430us baseline.
"""

import sys

for _p in ("/opt/trn_rl_repo",):
    if _p not in sys.path:
        sys.path.insert(0, _p)

import numpy as np

import concourse.bass as bass
import concourse.mybir as mybir
from concourse.bass_utils import run_bass_kernel_spmd

S = 2048
D = 64
N_CORES = 8
ROWS_PER_CORE = S // N_CORES          # 256 output rows/core, 512KB each
F = 8192                              # f32 per 32KB chunk; value = 16 chunks
SUBROWS = ROWS_PER_CORE * 16          # 4096 32KB sub-rows per shard
N_BLOCKS = SUBROWS // 128             # 32 store blocks of 128 sub-rows

N_LIGHT = 13                          # blocks stored 124-wide (port-15 relief)
LIGHT_BLOCKS = list(range(N_LIGHT))
FULL_BLOCKS = list(range(N_LIGHT, N_BLOCKS))
# Makeup piece k covers light block k's sub-rows [128k+124, 128k+128)
# from mtile partitions [4i, 4i+4), i = MAKEUP_PIECE_IDS[k].  i in 0..7
# lands on even ports 0,2,..,14; i in 16..20 on odd ports 1,3,5,7,9 —
# 13 distinct ports, none of them port 15.
MAKEUP_PIECE_IDS = list(range(8)) + list(range(16, 21))
N_MVAL = 52                           # mtile rows shipped from host

TRACE = False          # test.py flips this to profile
TRACE_KWARGS = {}
LAST_RESULT = None     # BassKernelResults of the last run (for test.py)


def build_program():
    nc = bass.Bass()
    val = nc.declare_dram_parameter("value", [128, F], mybir.dt.float32,
                                    isOutput=False)
    mval = nc.declare_dram_parameter("mval", [N_MVAL, F], mybir.dt.float32,
                                     isOutput=False)
    out = nc.declare_dram_parameter("out", [SUBROWS, F], mybir.dt.float32,
                                    isOutput=True)

    vtile = nc.alloc_sbuf_tensor("vtile", [128, F], mybir.dt.float32)
    mtile = nc.alloc_sbuf_tensor("mtile", [128, F], mybir.dt.float32)

    main_blocks = LIGHT_BLOCKS + FULL_BLOCKS
    qa_main = main_blocks[0::2]
    qb_main = main_blocks[1::2]

    n_stores = N_BLOCKS + len(MAKEUP_PIECE_IDS)
    total_s = 16 * n_stores

    def emit_main(e, blk, ssem):
        a = 128 * blk
        if blk < N_LIGHT:
            e.dma_start(out=out[a:a + 124, :],
                        in_=vtile[0:124, :]).then_inc(ssem, 16)
        else:
            e.dma_start(out=out[a:a + 128, :],
                        in_=vtile[:, :]).then_inc(ssem, 16)

    with nc.Block() as block, \
         nc.semaphore("lsem") as lsem, \
         nc.semaphore("msem") as msem, \
         nc.semaphore("ssem") as ssem:

        @block.sync
        def _(sync):
            sync.dma_start(out=vtile[:, :], in_=val[:, :]).then_inc(lsem, 16)
            sync.wait_ge(lsem, 16)
            for blk in qa_main:
                emit_main(sync, blk, ssem)
            sync.wait_ge(ssem, total_s)

        @block.scalar
        def _(scalar):
            scalar.dma_start(out=mtile[0:32, :],
                             in_=mval[0:32, :]).then_inc(msem, 16)
            scalar.dma_start(out=mtile[64:84, :],
                             in_=mval[32:52, :]).then_inc(msem, 16)
            scalar.wait_ge(msem, 32)
            for k, piece in enumerate(MAKEUP_PIECE_IDS):
                a = 128 * LIGHT_BLOCKS[k] + 124
                scalar.dma_start(
                    out=out[a:a + 4, :],
                    in_=mtile[4 * piece:4 * piece + 4, :],
                ).then_inc(ssem, 16)
            scalar.wait_ge(lsem, 16)
            for blk in qb_main:
                emit_main(scalar, blk, ssem)
            scalar.wait_ge(ssem, total_s)

    return nc


def kernel(query=None, key=None, value=None, attn_mask=None, **_ignored):
    global LAST_RESULT
    value = np.ascontiguousarray(np.asarray(value, dtype=np.float32))
    vflat = value.reshape(16, F)                      # 16 chunks of 32KB
    vexp = np.ascontiguousarray(np.tile(vflat, (8, 1)))   # [128, F]
    midx = 12 + (np.arange(N_MVAL) % 4)               # chunks 12..15 cyclic
    mval = np.ascontiguousarray(vflat[midx])          # [52, F]

    nc = build_program()
    core_ids = list(range(N_CORES))
    in_maps = [{"value": vexp, "mval": mval} for _ in core_ids]
    res = run_bass_kernel_spmd(nc, in_maps, core_ids, trace=TRACE,
                               **TRACE_KWARGS)
    LAST_RESULT = res

    # Core i supplies output rows [i*256, (i+1)*256).
    shards = [res.results[i]["out"].reshape(ROWS_PER_CORE, S, 1, D)
              for i in range(N_CORES)]
    return np.concatenate(shards, axis=0)


# revision 3
# speedup vs baseline: 2.4970x; 2.4970x over previous
"""Trainium2 Bass kernel for nn_Model_39676907882504.

Math: qk = (q @ k^T)/8 has shape [1,2048,1,1]; after the transposes it is
[2048,1,1,1], and softmax over the trailing size-1 axis is exactly 1.0
regardless of qk (exp(x-max)/sum == 1/1 bit-exactly). The final matmul
[S,Q,B,Q] @ [B,S,Q,D] with attn_weight == 1 therefore reduces to
broadcasting `value` across a new leading dim:

    output[i, j, 0, :] = value[0, j, 0, :]   for all i in [0, 2048)

i.e. a 512KB -> 1GiB broadcast copy.  Pure memory-regime kernel.

Sharding (per the hint): leading output dim (2048 rows) split across the
8 cores, 256 rows/core (= 4096 sub-rows of 32KB; sub-row d holds value
chunk d%16); value replicated.

Per-core plan:

  * One 4MiB load of a host-pre-tiled value [128, 8192] (partition q =
    chunk q%16).
  * Trace analysis: every dma_start costs SDMA engine 15 ~2.7us of
    completion-stall (inflated packets right after each instruction's
    semaphore descriptor), so 32 stores lost ~90us on engine 15 alone.
    Fix: TWO giant store instructions (one per HW-DGE queue), each 64MiB
    = 2048 descriptors of 32KB.  The SBUF side re-reads the same 4MiB
    via a stride-0 middle dim [128, 16, 8192]; the DRAM side is the
    transposed view out.rearrange("(c q) e -> q c e") so descriptor
    (q, c) lands at sub-row q + 128*c, whose content is chunk q%16.
  * Descriptors split across the 16 SDMA engines as contiguous runs of
    the outer (partition) dim -> 8 partitions x 16 reps = 128 descs per
    engine per instruction, all at the 32KB line rate of ~27 GB/s.

Predicted span: 9.6us load + 256 descs/engine x 1.216us ~ 322us.
"""

import sys

for _p in ("/opt/trn_rl_repo",):
    if _p not in sys.path:
        sys.path.insert(0, _p)

import numpy as np

import concourse.bass as bass
import concourse.mybir as mybir
from concourse.bass_utils import run_bass_kernel_spmd

S = 2048
D = 64
N_CORES = 8
ROWS_PER_CORE = S // N_CORES          # 256 output rows/core, 512KB each
F = 8192                              # f32 per 32KB chunk; value = 16 chunks
SUBROWS = ROWS_PER_CORE * 16          # 4096 32KB sub-rows per shard
NREP = SUBROWS // 128                 # 32 broadcast reps of the 128-part tile

TRACE = False          # test.py flips this to profile
TRACE_KWARGS = {}
LAST_RESULT = None     # BassKernelResults of the last run (for test.py)


def build_program():
    nc = bass.Bass()
    val = nc.declare_dram_parameter("value", [128, F], mybir.dt.float32,
                                    isOutput=False)
    out = nc.declare_dram_parameter("out", [SUBROWS, F], mybir.dt.float32,
                                    isOutput=True)

    vtile = nc.alloc_sbuf_tensor("vtile", [128, F], mybir.dt.float32)

    # [q, c, e]: sub-row q + 128*c <- vtile partition q (chunk q%16), so
    # every sub-row d gets chunk d%16.  Split the c axis across queues.
    out_qce = out[:, :].rearrange("(c q) e -> q c e", q=128)
    half = NREP // 2

    def in_bcast(reps):
        return vtile[:, :].unsqueeze(1).broadcast_to((128, reps, F))

    with nc.Block() as block, \
         nc.semaphore("lsem") as lsem, \
         nc.semaphore("ssem") as ssem:

        @block.sync
        def _(sync):
            sync.dma_start(out=vtile[:, :], in_=val[:, :]).then_inc(lsem, 16)
            sync.wait_ge(lsem, 16)
            sync.dma_start(out=out_qce[:, 0:half, :],
                           in_=in_bcast(half)).then_inc(ssem, 16)
            sync.wait_ge(ssem, 32)

        @block.scalar
        def _(scalar):
            scalar.wait_ge(lsem, 16)
            scalar.dma_start(out=out_qce[:, half:NREP, :],
                             in_=in_bcast(NREP - half)).then_inc(ssem, 16)
            scalar.wait_ge(ssem, 32)

    return nc


def kernel(query=None, key=None, value=None, attn_mask=None, **_ignored):
    global LAST_RESULT
    value = np.ascontiguousarray(np.asarray(value, dtype=np.float32))
    vflat = value.reshape(16, F)                      # 16 chunks of 32KB
    vexp = np.ascontiguousarray(np.tile(vflat, (8, 1)))   # [128, F]

    nc = build_program()
    core_ids = list(range(N_CORES))
    in_maps = [{"value": vexp} for _ in core_ids]
    res = run_bass_kernel_spmd(nc, in_maps, core_ids, trace=TRACE,
                               **TRACE_KWARGS)
    LAST_RESULT = res

    # Core i supplies output rows [i*256, (i+1)*256).
    shards = [res.results[i]["out"].reshape(ROWS_PER_CORE, S, 1, D)
              for i in range(N_CORES)]
    return np.concatenate(shards, axis=0)
